# revision 5
# baseline (speedup 1.0000x reference)
"""Trainium2 Bass kernel for a teacher-forced GRU decoder (nn_DecoderRNN).

Reference semantics (B=64, T=64, H=1024, V=32000):
    tokens = [SOS, target[:, :-1]]
    h_0 = encoder_hidden[0]
    per step: x = relu(emb[tok]); gi = x@w_ih.T + b_ih; gh = h@w_hh.T + b_hh
              r = sig(i_r+h_r); z = sig(i_z+h_z); n = tanh(i_n + r*h_n)
              h = (1-z)*n + z*h;  logits_t = h@out_w.T + out_b
    out = log_softmax(logits, -1), h_final

Distribution across 8 NeuronCores:
  - gi precompute: data-parallel over batch (each core 8 batch rows, all T),
    exchanged with one AllGather.
  - GRU recurrence: replicated over all cores (full B=64): per-step cost is
    streaming w_hh through the PE, which batch-sharding cannot reduce, and
    per-step collectives (~5-10us floor) are too slow to shard the gate dim.
  - Output projection + log_softmax: vocab-sharded (4000 rows of out_w per
    core, SBUF-resident). Projection matmuls are interleaved into the
    recurrence's PE gate-wait gaps. The log-softmax denominator is combined
    across cores with 16 tiny AllReduces, pipelined behind the compute.

All matmuls in bf16 with f32 PSUM accumulation; carried hidden state and gate
math in f32 (numpy-emulated max error vs f32 reference ~1e-3).
"""

import numpy as np
import ml_dtypes

import concourse.bass as bass
import concourse.bacc as bacc
import concourse.mybir as mybir
import concourse.tile as tile
from concourse.bass_utils import run_bass_kernel_spmd
from concourse.masks import make_identity

SOS_TOKEN = 0
F32 = mybir.dt.float32
BF16 = mybir.dt.bfloat16
AF = mybir.ActivationFunctionType
ALU = mybir.AluOpType


class Cfg:
    def __init__(self, B=64, T=64, H=1024, V=32000, NC=8,
                 has_bgi=False, has_bhn=False, has_ob=False):
        assert B == 64 and H % 1024 == 0 and T % 2 == 0 and V % NC == 0
        self.B, self.T, self.H, self.V, self.NC = B, T, H, V, NC
        self.KT = H // 128               # k-tiles over H
        self.BL = B // NC                # local batch rows (gi sharding)
        self.BTL = T * self.BL           # local (t-major) bt rows for gi
        self.H3 = 3 * H
        self.Vloc = V // NC
        vc = min(500, self.Vloc)         # vocab chunk (<=512 f32 psum bank)
        while self.Vloc % vc:
            vc -= 1
        self.VC = vc
        self.NVC = self.Vloc // vc
        self.HS = H // 512               # 512-wide H slices
        self.NT = B * T // 128           # logits bt tiles
        self.has_bgi, self.has_bhn, self.has_ob = has_bgi, has_bhn, has_ob


def build_nc(cfg: Cfg):
    nc = bacc.Bacc("TRN2", target_bir_lowering=False, debug=False,
                   enable_asserts=False, num_devices=cfg.NC)
    B, T, H, H3, Vloc = cfg.B, cfg.T, cfg.H, cfg.H3, cfg.Vloc
    io = {}
    io["xt"] = nc.dram_tensor("xt", [H, cfg.BTL], BF16, kind="ExternalInput").ap()
    io["h0"] = nc.dram_tensor("h0", [B, H], F32, kind="ExternalInput").ap()
    io["h0t"] = nc.dram_tensor("h0t", [H, B], BF16, kind="ExternalInput").ap()
    io["wih_t"] = nc.dram_tensor("wih_t", [H, H3], BF16, kind="ExternalInput").ap()
    io["whh_t"] = nc.dram_tensor("whh_t", [H, H3], BF16, kind="ExternalInput").ap()
    io["owt_t"] = nc.dram_tensor("owt_t", [H, Vloc], BF16, kind="ExternalInput").ap()
    io["bgi"] = nc.dram_tensor("bgi", [1, H3], F32, kind="ExternalInput").ap() if cfg.has_bgi else None
    io["bhn"] = nc.dram_tensor("bhn", [1, H], F32, kind="ExternalInput").ap() if cfg.has_bhn else None
    io["ob"] = nc.dram_tensor("ob", [1, Vloc], F32, kind="ExternalInput").ap() if cfg.has_ob else None
    io["outp"] = nc.dram_tensor("outp", [B, T, Vloc], F32, kind="ExternalOutput").ap()
    io["hfin"] = nc.dram_tensor("hfin", [B, H], F32, kind="ExternalOutput").ap()

    with tile.TileContext(nc) as tc:
        with tc.tile_pool(name="dram", bufs=1, space="DRAM") as dram, \
             tc.tile_pool(name="sb", bufs=1) as sb, \
             tc.tile_pool(name="ps", bufs=1, space="PSUM") as ps:
            _body(nc, tc, cfg, dram, sb, ps, io)
    nc.compile()
    return nc


def _body(nc, tc, cfg, dram, sb, ps, io):
    B, T, H, KT, H3 = cfg.B, cfg.T, cfg.H, cfg.KT, cfg.H3
    Vloc, VC, NVC, HS, NT = cfg.Vloc, cfg.VC, cfg.NVC, cfg.HS, cfg.NT
    BL, BTL, NC = cfg.BL, cfg.BTL, cfg.NC
    BTT = BTL // 128                      # local bt tiles in phase A
    xt, h0, h0t = io["xt"], io["h0"], io["h0t"]
    wih_t, whh_t, owt_t = io["wih_t"], io["whh_t"], io["owt_t"]
    bgi, bhn, ob = io["bgi"], io["bhn"], io["ob"]
    outp, hfin = io["outp"], io["hfin"]

    # ---- resident SBUF ----------------------------------------------------
    whh_sb = sb.tile([128, KT * H3], BF16, tag="whh")     # k-major moving w_hh.T
    owt_sb = sb.tile([128, KT * Vloc], BF16, tag="owt")   # k-major moving out_w.T
    h0t_sb = sb.tile([128, KT * B], BF16, tag="h0t")
    h_cur = sb.tile([B, H], F32, tag="hcur")
    ident = sb.tile([128, 128], BF16, tag="ident")

    for k in range(KT):
        nc.sync.dma_start(whh_sb[:, k * H3:(k + 1) * H3],
                          whh_t[k * 128:(k + 1) * 128, :])
        nc.sync.dma_start(owt_sb[:, k * Vloc:(k + 1) * Vloc],
                          owt_t[k * 128:(k + 1) * 128, :])
        nc.sync.dma_start(h0t_sb[:, k * B:(k + 1) * B],
                          h0t[k * 128:(k + 1) * 128, :])
    nc.sync.dma_start(h_cur[:, :], h0[:, :])
    make_identity(nc, ident[:, :])

    bgi_rep = bhn_rep = ob_rep = None
    if bgi is not None:
        bgi_rep = sb.tile([128, H3], F32, tag="bgi")
        nc.sync.dma_start(bgi_rep[0:1, :], bgi[:, :])
        nc.gpsimd.partition_broadcast(bgi_rep[:, :], bgi_rep[0:1, :])
    if bhn is not None:
        bhn_rep = sb.tile([128, H], F32, tag="bhn")
        nc.sync.dma_start(bhn_rep[0:1, :], bhn[:, :])
        nc.gpsimd.partition_broadcast(bhn_rep[:, :], bhn_rep[0:1, :])
    if ob is not None:
        ob_rep = sb.tile([128, Vloc], F32, tag="ob")
        nc.sync.dma_start(ob_rep[0:1, :], ob[:, :])
        nc.gpsimd.partition_broadcast(ob_rep[:, :], ob_rep[0:1, :])

    gi_in = dram.tile([BTL, H3], BF16, tag="gi_in")
    gi_all = dram.tile([BTL * NC, H3], BF16, tag="gi_all", addr_space="Shared")

    # ---- phase A: GI = relu(x) @ w_ih.T (+ bias) for local batch rows -----
    xt_r = xt.rearrange("(k p) n -> p k n", p=128)
    xtiles = []
    for m in range(BTT):
        xtile = sb.tile([128, KT * 128], BF16, tag="raw", bufs=4, name=f"xtile{m}")
        nc.sync.dma_start(xtile[:, :], xt_r[:, :, m * 128:(m + 1) * 128])
        nc.vector.tensor_scalar_max(xtile[:, :], xtile[:, :], 0.0)
        xtiles.append(xtile)

    NCH = H3 // 512
    for ch in range(NCH):
        pss = [ps.tile([128, 512], F32, tag="mm", bufs=5, name=f"gips{ch}_{m}")
               for m in range(BTT)]
        for k in range(KT):
            wt = sb.tile([128, 512], BF16, tag="kb1", bufs=6, name=f"wt{ch}_{k}")
            nc.sync.dma_start(wt[:, :],
                              wih_t[k * 128:(k + 1) * 128, ch * 512:(ch + 1) * 512])
            for m in range(BTT):
                nc.tensor.matmul(pss[m][:, :],
                                 xtiles[m][:, k * 128:(k + 1) * 128],
                                 wt[:, :],
                                 start=(k == 0), stop=(k == KT - 1))
        for m in range(BTT):
            gia = sb.tile([128, 512], BF16, tag="kb1", bufs=6, name=f"gia{ch}_{m}")
            if bgi_rep is not None:
                nc.vector.tensor_tensor(gia[:, :], pss[m][:, :],
                                        bgi_rep[:, ch * 512:(ch + 1) * 512], ALU.add)
            else:
                nc.vector.tensor_copy(gia[:, :], pss[m][:, :])
            nc.sync.dma_start(
                gi_in[m * 128:(m + 1) * 128, ch * 512:(ch + 1) * 512], gia[:, :])

    # ---- AllGather gi across cores ---------------------------------------
    nc.gpsimd.collective_compute(
        "AllGather", ALU.bypass,
        replica_groups=[list(range(NC))],
        ins=[gi_in.opt()], outs=[gi_all.opt()],
    )
    gi_view = gi_all.rearrange("(c t b) g -> t c b g", c=NC, t=T, b=BL)

    # ---- main loop: GRU steps + interleaved logits chunks ----------------
    work = [(j, vc) for j in range(NT) for vc in range(NVC)]
    wq = 0
    h_grps = [None] * (NT + 2)
    raw_tiles = [None] * NT
    es_parts = [None] * NT
    ar_stage = [None] * (NT // 2 + 1)
    chunks_emitted = [0] * NT
    ar_bufs = [(dram.tile([128, 2], F32, tag=f"ar_in{m % 4}", name=f"arin{m}"),
                dram.tile([128, 2], F32, tag=f"ar_out{m % 4}", name=f"arout{m}",
                          addr_space="Shared"))
               for m in range(NT // 2)]
    ov = outp.rearrange("b (j s) v -> j s b v", s=2)

    def emit_logits_chunk(j, vc):
        if raw_tiles[j] is None:
            raw_tiles[j] = sb.tile([128, Vloc], BF16, tag="raw", bufs=4,
                                   name=f"raw{j}")
            es_parts[j] = sb.tile([128, NVC], F32, tag="es", bufs=3,
                                  name=f"es{j}")
        raw = raw_tiles[j]
        pl = ps.tile([128, VC], F32, tag="lg", bufs=2, name=f"pl{j}_{vc}")
        for k in range(KT):
            nc.tensor.matmul(
                pl[:, :],
                h_grps[j][:, k * 128:(k + 1) * 128],
                owt_sb[:, k * Vloc + vc * VC: k * Vloc + (vc + 1) * VC],
                start=(k == 0), stop=(k == KT - 1))
        if ob_rep is not None:
            nc.vector.tensor_tensor(raw[:, vc * VC:(vc + 1) * VC], pl[:, :],
                                    ob_rep[:, vc * VC:(vc + 1) * VC], ALU.add)
        else:
            nc.vector.tensor_copy(raw[:, vc * VC:(vc + 1) * VC], pl[:, :])
        scr = sb.tile([128, VC], BF16, tag="kb1", bufs=6, name=f"scr{j}_{vc}")
        nc.scalar.activation(scr[:, :], raw[:, vc * VC:(vc + 1) * VC], AF.Exp,
                             accum_out=es_parts[j][:, vc:vc + 1])
        chunks_emitted[j] += 1
        if chunks_emitted[j] == NVC:
            finish_tile(j)

    def finish_tile(j):
        m = j // 2
        if j % 2 == 0:
            ar_stage[m] = sb.tile([128, 2], F32, tag="arst", bufs=2,
                                  name=f"arst{m}")
        nc.vector.tensor_reduce(ar_stage[m][:, j % 2:j % 2 + 1],
                                es_parts[j][:, :], mybir.AxisListType.X, ALU.add)
        if j % 2 == 1:
            emit_ar_and_sub(m)

    def emit_ar_and_sub(m):
        ar_in, ar_out = ar_bufs[m]
        nc.sync.dma_start(ar_in[:, :], ar_stage[m][:, :])
        nc.gpsimd.collective_compute(
            "AllReduce", ALU.add,
            replica_groups=[list(range(NC))],
            ins=[ar_in.opt()], outs=[ar_out.opt()],
        )
        es_sum = sb.tile([128, 2], F32, tag="lse_raw", bufs=2, name=f"essum{m}")
        nc.sync.dma_start(es_sum[:, :], ar_out[:, :])
        lse = sb.tile([128, 2], F32, tag="lse", bufs=2, name=f"lse{m}")
        nc.scalar.activation(lse[:, :], es_sum[:, :], AF.Ln)
        qs = Vloc // 4 if Vloc % 4 == 0 else Vloc
        nq = Vloc // qs
        for j in (2 * m, 2 * m + 1):
            for q in range(nq):
                outst = sb.tile([128, qs], F32, tag="outst", bufs=2,
                                name=f"outst{j}_{q}")
                nc.vector.tensor_scalar(
                    outst[:, :], raw_tiles[j][:, q * qs:(q + 1) * qs],
                    lse[:, j % 2:j % 2 + 1], None, ALU.subtract)
                nc.sync.dma_start(ov[j, :, :, q * qs:(q + 1) * qs], outst[:, :])

    rz_sb = sb.tile([B, 2 * H], F32, tag="rz")
    n_sb = sb.tile([B, H], F32, tag="n")
    hnew_bf = sb.tile([B, H], BF16, tag="hnew")

    for t in range(T):
        g = t // 2
        if t % 2 == 0:
            h_grps[g] = sb.tile([128, KT * 128], BF16, tag="hgrp", bufs=6,
                                name=f"hgrp{g}")
        gi_sb = sb.tile([B, H3], BF16, tag="gi", bufs=2, name=f"gi{t}")
        nc.sync.dma_start(gi_sb[:, :], gi_view[t])

        for s in range(HS):
            sl = slice(s * 512, s * 512 + 512)
            pss = []
            for gate in range(3):
                p = ps.tile([B, 512], F32, tag="mm", bufs=5, name=f"gh{t}_{s}_{gate}")
                for k in range(KT):
                    if t == 0:
                        lhsT = h0t_sb[:, k * B:(k + 1) * B]
                    else:
                        gp, sp = (t - 1) // 2, (t - 1) % 2
                        lhsT = h_grps[gp][:, k * 128 + sp * 64: k * 128 + sp * 64 + 64]
                    co = k * H3 + gate * H + s * 512
                    nc.tensor.matmul(p[:, :], lhsT, whh_sb[:, co:co + 512],
                                     start=(k == 0), stop=(k == KT - 1))
                pss.append(p)
            pr, pz, pn = pss
            # r, z gates
            nc.vector.tensor_tensor(pr[:, :], pr[:, :],
                                    gi_sb[:, s * 512:s * 512 + 512], ALU.add)
            nc.vector.tensor_tensor(pz[:, :], pz[:, :],
                                    gi_sb[:, H + s * 512:H + s * 512 + 512], ALU.add)
            nc.scalar.activation(rz_sb[:, s * 512:s * 512 + 512], pr[:, :], AF.Sigmoid)
            nc.scalar.activation(rz_sb[:, H + s * 512:H + s * 512 + 512], pz[:, :], AF.Sigmoid)
            # n = tanh(gi_n + r*(gh_n [+ bhn]))
            if bhn_rep is not None:
                nc.vector.tensor_tensor(pn[:, :], pn[:, :], bhn_rep[0:B, sl], ALU.add)
            nc.vector.tensor_tensor(pn[:, :], pn[:, :], rz_sb[:, sl], ALU.mult)
            nc.vector.tensor_tensor(pn[:, :], pn[:, :],
                                    gi_sb[:, 2 * H + s * 512:2 * H + s * 512 + 512], ALU.add)
            nc.scalar.activation(n_sb[:, sl], pn[:, :], AF.Tanh)
            # h = n + z*(h-n); d reuses the dead r slice of rz_sb
            dsl = rz_sb[:, s * 512:s * 512 + 512]
            nc.vector.tensor_tensor(dsl, h_cur[:, sl], n_sb[:, sl], ALU.subtract)
            nc.vector.tensor_tensor(dsl, dsl, rz_sb[:, H + s * 512:H + s * 512 + 512], ALU.mult)
            nc.vector.tensor_tensor(h_cur[:, sl], n_sb[:, sl], dsl, ALU.add)
            nc.scalar.activation(hnew_bf[:, sl], h_cur[:, sl], AF.Copy)

        # interleave logits chunks into the PE gate-wait gap
        if t >= 2:
            for _ in range(max(1, NVC // 2)):
                if wq < len(work) and work[wq][0] <= (t - 2) // 2:
                    emit_logits_chunk(*work[wq])
                    wq += 1

        # transpose h_new into h_grp[g] columns (t%2)*64
        for kp in range(KT // 2):
            tr = ps.tile([128, 128], BF16, tag="tr", bufs=1, name=f"tr{t}_{kp}")
            for i in range(2):
                k = 2 * kp + i
                nc.tensor.transpose(tr[:, i * 64:(i + 1) * 64],
                                    hnew_bf[:, k * 128:(k + 1) * 128],
                                    ident[0:B, 0:B])
            for i in range(2):
                k = 2 * kp + i
                co = k * 128 + (t % 2) * 64
                nc.vector.tensor_copy(h_grps[g][:, co:co + 64],
                                      tr[:, i * 64:(i + 1) * 64])

    while wq < len(work):
        emit_logits_chunk(*work[wq])
        wq += 1

    nc.sync.dma_start(hfin[:, :], h_cur[:, :])


# --------------------------------------------------------------------------
# host-side entry
# --------------------------------------------------------------------------
_BUILD_CACHE = {}


def _bf16(x):
    return np.ascontiguousarray(x.astype(ml_dtypes.bfloat16))


def kernel(encoder_outputs, encoder_hidden, target, emb, w_ih, w_hh, b_ih, b_hh,
           out_w, out_b):
    target = np.asarray(target)
    emb = np.asarray(emb, dtype=np.float32)
    w_ih = np.asarray(w_ih, dtype=np.float32)
    w_hh = np.asarray(w_hh, dtype=np.float32)
    b_ih = np.asarray(b_ih, dtype=np.float32)
    b_hh = np.asarray(b_hh, dtype=np.float32)
    out_w = np.asarray(out_w, dtype=np.float32)
    out_b = np.asarray(out_b, dtype=np.float32)
    h0 = np.asarray(encoder_hidden, dtype=np.float32)[0]      # [B, H]
    B, T = target.shape
    V, H = emb.shape
    NC = 8

    has_bgi = bool(np.any(b_ih != 0) or np.any(b_hh[:2 * H] != 0))
    has_bhn = bool(np.any(b_hh[2 * H:] != 0))
    has_ob = bool(np.any(out_b != 0))

    cfg = Cfg(B=B, T=T, H=H, V=V, NC=NC,
              has_bgi=has_bgi, has_bhn=has_bhn, has_ob=has_ob)
    key = (B, T, H, V, NC, has_bgi, has_bhn, has_ob)
    if key not in _BUILD_CACHE:
        _BUILD_CACHE[key] = build_nc(cfg)
    nc = _BUILD_CACHE[key]

    tokens = np.concatenate(
        [np.full((B, 1), SOS_TOKEN, dtype=target.dtype), target[:, :-1]], axis=1)

    wih_tb = _bf16(w_ih.T)
    whh_tb = _bf16(w_hh.T)
    h0tb = _bf16(h0.T)
    bgi_v = (b_ih + np.concatenate([b_hh[:2 * H], np.zeros(H, np.float32)]))[None]
    bhn_v = np.ascontiguousarray(b_hh[2 * H:][None])

    BL = B // NC
    in_maps = []
    for c in range(NC):
        toks = np.asarray(tokens[c * BL:(c + 1) * BL], dtype=np.int64)
        xs = emb[toks]                                # [BL, T, H] gather
        xt_c = _bf16(np.transpose(xs, (2, 1, 0)).reshape(H, T * BL))
        owt_c = _bf16(out_w[c * cfg.Vloc:(c + 1) * cfg.Vloc].T)
        m = {"xt": xt_c, "h0": h0, "h0t": h0tb,
             "wih_t": wih_tb, "whh_t": whh_tb, "owt_t": owt_c}
        if has_bgi:
            m["bgi"] = np.ascontiguousarray(bgi_v)
        if has_bhn:
            m["bhn"] = bhn_v
        if has_ob:
            m["ob"] = np.ascontiguousarray(out_b[c * cfg.Vloc:(c + 1) * cfg.Vloc][None])
        in_maps.append(m)

    global _last_in_maps
    _last_in_maps = in_maps
    res = run_bass_kernel_spmd(nc, in_maps, core_ids=list(range(NC)))
    outp = np.concatenate([res.results[c]["outp"] for c in range(NC)], axis=2)
    h_final = res.results[0]["hfin"][None]
    return outp, h_final


_last_in_maps = None
